# revision 59
# baseline (speedup 1.0000x reference)
"""Trainium2 Bass kernel for CausalGatedD2Attention.

Math (per batch b):
  xn   = LayerNorm(x) * ln_g + ln_b            [T, D]
  qkv  = xn @ qkv_w + qkv_b                     -> q, k, v  [T, D] each
  gate = sigmoid(xn @ gate_w + gate_b)
  k    = elu(k * gate) + 1 ;  q = elu(q) + 1
  attn = tril(q @ k^T)                          [T, T]
  out  = (attn @ v) / (rowsum(attn) + eps)      [T, D]

Sharding: 4 batches x 2 cores.  Within a pair, core parity par in {0,1}
owns the even/odd 128-row t-chunks of its batch (balances the causal
triangle).  Each core receives its batch CHUNK-PERMUTED: its own-parity
chunks first (slots 0..7 = local t-chunks), the complementary chunks
last (slots 8..15).  With that layout one uniform program serves all
cores:
  - q rows are always slots 0..7 (the first half),
  - k/v cover all 16 slots,
  - causality becomes slot-triangular: t-chunk i attends s-slots
    {j : j%8 <= i}; only the two boundary diagonals depend on parity
    and those are handled by two host-provided [128,128] mask tiles
    (triangle for j<8, all-ones/all-zeros for j>=8).

Weights / biases are identical on every call, so they are baked into
the program as Const (inline) tensors: they ship inside the compiled
executable once instead of being re-uploaded per execution.  Only the
fp16 activations x (and two tiny mask tiles) move host->device per
call, and only the fp16 output moves back.  Compute accumulates in
fp32 PSUM.  ln_g / ln_b are folded into the projection weights on the
host, so the device LN is just (x - mean) * rsqrt(var + eps).

The denominator comes for free: v gets an appended ones-column, so
attn @ v_aug yields [num | den] in one accumulation.
"""

import sys

sys.path.insert(0, "/opt/trn_rl_repo")

import numpy as np

B, T, D = 4, 2048, 1024
P = 128
KD = D // P          # 8 contraction chunks
NT = T // P          # 16 chunk slots
NL = NT // 2         # 8 local t-chunks per core
LN_EPS = 1e-5
DEN_EPS = 1e-6
N_CORES = 8

_CACHE = {}


def _patched_tc(tile_mod):
    import bass_rust as _br
    from concourse.vector_clock import ScopedClock

    class TC(tile_mod.TileContext):
        """TileContext whose final drain splits sem waits one per
        instruction (walrus CoreV3 allows a single wait on Drain)."""

        def _spread_waits(self):
            # walrus allows at most 2 sem waits on engine instructions and
            # only 1 on CTRL-class ones (Drain/NoOp); Tile's scheduler can
            # emit more.  Move excess waits onto same-engine nops placed
            # immediately before the over-limit instruction.
            nc = self.nc
            for fnbb in nc.m.functions[0].blocks:
                insts = list(fnbb.instructions)
                out = []
                for inst in insts:
                    si = inst.sync_info
                    waits = list(si.on_wait) if si is not None else []
                    limit = 1
                    if len(waits) > limit:
                        excess = waits[limit:]
                        si.on_wait = waits[:limit]
                        inst.sync_info = si
                        for w in excess:
                            nop = nc.engines[inst.engine].nop(
                                nofuse=True, hint="wait_spread"
                            )
                            nop.ins.sync_info = _br.SyncInfo(
                                on_wait=[w], on_update=[]
                            )
                            # remove from wherever it was appended
                            for b2 in nc.m.functions[0].blocks:
                                cur = list(b2.instructions)
                                if cur and cur[-1] is nop.ins:
                                    b2.instructions = cur[:-1]
                                    break
                            out.append(nop.ins)
                    out.append(inst)
                fnbb.instructions = out

        def _drain_and_barrier(self, tick_clock, wait_clock):
            self._spread_waits()
            drain_inst = self.nc.sync.drain()
            wait_clock.add_sem_waits(
                drain_inst.ins, ScopedClock({None: tick_clock.global_clock})
            )
            si = drain_inst.ins.sync_info
            waits = list(si.on_wait)
            if len(waits) > 1:
                si.on_wait = waits[:1]
                drain_inst.ins.sync_info = si
                for i in range(1, len(waits)):
                    nop = self.nc.sync.nop(nofuse=True, hint="drain_extra_waits")
                    nop.ins.sync_info = _br.SyncInfo(
                        on_wait=waits[i : i + 1], on_update=[]
                    )
            self.nc.all_engine_barrier()
            assert self.sems is not None
            popped = self.nc._tile_sem_poison_stack.pop()
            assert popped is self._sem_poison
            self.nc.clear_and_free_semaphores(list(self.sems.allocated().values()))
            self.nc.all_engine_barrier()

    return TC


def build_program(prec="f16", consts=None):
    import concourse.bass as bass
    import concourse.tile as tile
    from concourse import mybir
    from concourse.masks import make_identity

    TC = _patched_tc(tile)
    f32 = mybir.dt.float32
    Act = mybir.ActivationFunctionType
    Alu = mybir.AluOpType

    if prec == "f16":
        fio = mybir.dt.float16   # host<->device payload dtype
        fmm = mybir.dt.float16   # matmul operand dtype
    else:
        fio = f32
        fmm = f32

    nc = bass.Bass(num_devices=N_CORES)
    x_in = nc.declare_dram_parameter("x", [NL * P, D], fio, isOutput=False)
    masks_in = nc.declare_dram_parameter("masks", [2, P, P], f32, isOutput=False)
    out_d = nc.declare_dram_parameter("out", [NL * P, D], fio, isOutput=True)
    # weights ride inside the executable as load-time constants
    wq_t = nc.inline_tensor(consts["wq_t"], name="wq_t")
    wk_t = nc.inline_tensor(consts["wk_t"], name="wk_t")
    wg_t = nc.inline_tensor(consts["wg_t"], name="wg_t")
    wv_t = nc.inline_tensor(consts["wv_t"], name="wv_t")
    bqkv = nc.inline_tensor(consts["bqkv"], name="bqkv")
    bg_in = nc.inline_tensor(consts["bg"], name="bg")

    with TC(nc) as tc:
        const = tc.alloc_tile_pool(name="const", bufs=1)
        ident = const.tile([P, P], fmm, tag="ident")
        make_identity(nc, ident)
        # =========== phase CC: pair AllGather of x =======================
        # Each core uploads only its own-parity 128-row chunks (slots in
        # ascending global order).  The pair AllGather yields the full
        # batch in [even-chunks | odd-chunks] slot order on both cores.
        # Triggered first (in two pipelined groups of GC chunks) so the
        # wire time hides under the local-q LN + q-projection below,
        # which only need the local x; the full-batch phases then
        # consume gathered groups as they land.
        GC = NL // 2  # chunks per gather group per rank
        dram = tc.alloc_tile_pool(name="ccdram", bufs=1, space="DRAM")
        xtloc = [
            dram.tile([KD, P, GC * P], fmm, tag=f"xtloc{g}", name=f"xtloc{g}")
            for g in range(2)
        ]
        xtall = [
            dram.tile([2, KD, P, GC * P], fmm, tag=f"xtall{g}", name=f"xtall{g}")
            for g in range(2)
        ]
        rgroups = [[2 * p, 2 * p + 1] for p in range(N_CORES // 2)]

        # biases: [P, KD] with column m = bias[m*128:(m+1)*128]
        bq_sb = const.tile([P, KD], f32, tag="bq")
        bk_sb = const.tile([P, KD], f32, tag="bk")
        bg_sb = const.tile([P, KD], f32, tag="bgs")
        # transposed bias loads are ~1k tiny descriptors each; keep them
        # off the sync DMA queue so the gather's bounce copy goes first
        b3 = bqkv.rearrange("(s m p) -> s m p", s=3, m=KD, p=P)
        nc.scalar.dma_start(out=bq_sb, in_=b3[0].rearrange("m p -> p m"))
        nc.scalar.dma_start(out=bk_sb, in_=b3[1].rearrange("m p -> p m"))
        nc.scalar.dma_start(
            out=bg_sb, in_=bg_in.rearrange("(m p) -> p m", m=KD, p=P)
        )
        ln_eps = const.tile([P, 1], f32, tag="lneps")
        nc.vector.memset(ln_eps, LN_EPS)
        onez_sb = const.tile([P, 2], f32, tag="onez")
        nc.vector.memset(onez_sb[:, 0:1], 1.0)
        nc.vector.memset(onez_sb[:, 1:2], 0.0)
        m0_sb = const.tile([P, P], f32, tag="m0sb")
        m1_sb = const.tile([P, P], f32, tag="m1sb")
        nc.scalar.dma_start(out=m0_sb, in_=masks_in[0])
        nc.scalar.dma_start(out=m1_sb, in_=masks_in[1])

        # ========= phase XQ: local layernorm + transpose -> xqnT =========
        # (for the q side; local rows only, independent of the gather)
        def ln_chunk(src_ap, dstT, c, xpool, spool, pspool):
            xt = xpool.tile([P, D], fio, tag="xt")
            nc.sync.dma_start(out=xt, in_=src_ap)
            stats = spool.tile([P, 2, 6], f32, tag="stats")
            xr = xt.rearrange("p (n f) -> p n f", n=2)
            for sg in range(2):
                nc.vector.bn_stats(out=stats[:, sg], in_=xr[:, sg])
            mv = spool.tile([P, 2], f32, tag="mv")
            nc.vector.bn_aggr(out=mv, in_=stats)
            rstd = spool.tile([P, 1], f32, tag="rstd")
            nc.scalar.activation(
                out=rstd, in_=mv[:, 1:2], func=Act.Sqrt, bias=ln_eps, scale=1.0
            )
            rstd2 = spool.tile([P, 1], f32, tag="rstd2")
            nc.vector.reciprocal(out=rstd2, in_=rstd)
            nmr = spool.tile([P, 1], f32, tag="nmr")
            nc.vector.tensor_scalar(
                out=nmr,
                in0=mv[:, 0:1],
                scalar1=rstd2,
                scalar2=-1.0,
                op0=Alu.mult,
                op1=Alu.mult,
            )
            xn = xpool.tile([P, D], fmm, tag="xn")
            nc.scalar.activation(
                out=xn, in_=xt, func=Act.Identity, bias=nmr, scale=rstd2
            )
            for k in range(KD):
                ps = pspool.tile([P, P], fmm, tag="psT")
                nc.tensor.transpose(
                    out=ps, in_=xn[:, k * P : (k + 1) * P], identity=ident
                )
                if k % 2 == 0:
                    nc.vector.tensor_copy(dstT[k][:, c * P : (c + 1) * P], ps)
                else:
                    nc.scalar.copy(out=dstT[k][:, c * P : (c + 1) * P], in_=ps)

        # xqnT is split per gather group so each group's pack (and its
        # AllGather of LN'd+transposed columns) can fire as soon as that
        # group's chunks are done; the partner's xnT arrives ready, so no
        # full-batch LN/transpose phase exists at all.
        xqnT_pool = tc.alloc_tile_pool(name="xqnT", bufs=1)
        xqnT_g = [
            [
                xqnT_pool.tile(
                    [P, GC * P], fmm, tag=f"xqnT{g}_{k}", name=f"xqnT{g}_{k}"
                )
                for k in range(KD)
            ]
            for g in range(2)
        ]
        xpool = tc.alloc_tile_pool(name="xqwork", bufs=3)
        spool = tc.alloc_tile_pool(name="xqstat", bufs=4)
        pspool = tc.alloc_tile_pool(name="psTq", bufs=4, space="PSUM")
        for g in range(2):
            for i in range(GC):
                c = g * GC + i
                ln_chunk(
                    x_in[c * P : (c + 1) * P, :],
                    xqnT_g[g],
                    i,
                    xpool,
                    spool,
                    pspool,
                )
            for k in range(KD):
                eng = nc.sync if k % 2 == 0 else nc.scalar
                eng.dma_start(out=xtloc[g][k], in_=xqnT_g[g][k])
            nc.gpsimd.collective_compute(
                "AllGather",
                mybir.AluOpType.bypass,
                replica_groups=rgroups,
                ins=[xtloc[g].opt()],
                outs=[xtall[g].opt()],
            )
        pspool.release()
        spool.release()
        xpool.release()

        # v tiles live SBUF-resident until the OUT phase; allocate their
        # pool first so right-side pools release in stack order
        # (kT, qT, vres).
        v_pool = tc.alloc_tile_pool(name="vres", bufs=1, side="right")
        vres = [
            v_pool.tile([P, D + 1], fmm, tag=f"v{s}", name=f"v{s}")
            for s in range(NT)
        ]

        # =========== phase QP: q projection -> qT (elu+1) ================
        # (also independent of the gather; elu min runs on DVE, not
        # gpsimd, so it cannot queue behind the collective)
        qT_pool = tc.alloc_tile_pool(name="qT", bufs=1, side="right")
        qT = [
            qT_pool.tile([P, NL * P], fmm, tag=f"qT{m}", name=f"qT{m}")
            for m in range(KD)
        ]
        wpool = tc.alloc_tile_pool(name="wq", bufs=4)
        epool = tc.alloc_tile_pool(name="qev", bufs=3)
        psq = tc.alloc_tile_pool(name="psQ", bufs=3, space="PSUM")
        for m in range(KD):
            ps = psq.tile([P, NL * P], f32, tag="psQ")
            for k in range(KD):
                wqt = wpool.tile([P, P], fmm, tag="wqt")
                nc.sync.dma_start(out=wqt, in_=wq_t[m, k])
                for sc in range(2):
                    nc.tensor.matmul(
                        out=ps[:, sc * 512 : (sc + 1) * 512],
                        lhsT=(wqt),
                        rhs=(xqnT_g[sc][k]),
                        start=(k == 0),
                        stop=(k == KD - 1),
                    )
            for sc in range(2):
                cols = slice(sc * 512, (sc + 1) * 512)
                qx = epool.tile([P, 512], f32, tag="qx")
                nc.scalar.activation(
                    out=qx,
                    in_=ps[:, cols],
                    func=Act.Identity,
                    bias=bq_sb[:, m : m + 1],
                    scale=1.0,
                )
                m0 = epool.tile([P, 512], f32, tag="qm0")
                nc.vector.tensor_scalar(
                    out=m0, in0=qx, scalar1=0.0, scalar2=None, op0=Alu.min
                )
                e = epool.tile([P, 512], f32, tag="qe")
                nc.scalar.activation(out=e, in_=m0, func=Act.Exp)
                nc.vector.scalar_tensor_tensor(
                    out=qT[m][:, cols],
                    in0=qx,
                    scalar=0.0,
                    in1=e,
                    op0=Alu.max,
                    op1=Alu.add,
                )
        psq.release()
        epool.release()
        wpool.release()
        xqnT_pool.release()

        # ====== phases XF/KG/V: consume gathered x group by group ========
        # Group g lands slots {g*GC..g*GC+GC-1} (even rank) and
        # {NL+g*GC..} (odd rank): LN+transpose them into xnT, then run
        # the k/gate projections for those 512-wide column windows and
        # the v projections for those slots, while the next gather
        # group is still in flight.
        # xnT as 4 window tiles per k (window w = slots {4w..4w+3}):
        # group g's unpack fills windows {g, 2+g} straight from the
        # gathered, already-LN'd-and-transposed columns
        xnT_pool = tc.alloc_tile_pool(name="xnT", bufs=1)
        xnT_w = [
            [
                xnT_pool.tile(
                    [P, GC * P], fmm, tag=f"xnT{w}_{k}", name=f"xnT{w}_{k}"
                )
                for k in range(KD)
            ]
            for w in range(4)
        ]
        wvpool = tc.alloc_tile_pool(name="wv", bufs=1)
        vb_sb = wvpool.tile([P, D], f32, tag="vb", name="vb_sb")
        vslice = b3[2].rearrange("m p -> (m p)")
        vb_bcast = bass.AP(
            tensor=vslice.tensor, offset=vslice.offset, ap=[[0, P], *vslice.ap]
        )
        nc.sync.dma_start(out=vb_sb, in_=vb_bcast)
        wv = []
        for k in range(KD):
            t = wvpool.tile([P, D], fmm, tag=f"wv{k}", name=f"wv{k}")
            nc.sync.dma_start(out=t, in_=wv_t[k])
            wv.append(t)
        kT_pool = tc.alloc_tile_pool(name="kT", bufs=1, side="right")
        kT = [
            kT_pool.tile([P, T], fmm, tag=f"kT{m}", name=f"kT{m}")
            for m in range(KD)
        ]
        for g in range(2):
            slots = [r * NL + g * GC + i for r in range(2) for i in range(GC)]
            # --- unpack_g: gathered xnT columns -> SBUF window tiles ---
            for k in range(KD):
                eng = nc.sync if k % 2 == 0 else nc.scalar
                eng.dma_start(out=xnT_w[g][k], in_=xtall[g][0, k])
                eng.dma_start(out=xnT_w[2 + g][k], in_=xtall[g][1, k])
            # --- KG_g: k/gate projections for this group's columns ---
            wpool = tc.alloc_tile_pool(name="wkg", bufs=8)
            epool = tc.alloc_tile_pool(name="kgev", bufs=2)
            pskg = tc.alloc_tile_pool(name="psKG", bufs=1, space="PSUM")
            psv = tc.alloc_tile_pool(name="psV", bufs=2, space="PSUM")
            scs = [g, 2 + g]  # 512-col windows of this group

            def v_slot(s):
                ps = psv.tile([P, D], f32, tag="psV", name="ps")
                for k in range(KD):
                    for dc in range(2):
                        nc.tensor.matmul(
                            out=ps[:, dc * 512 : (dc + 1) * 512],
                            lhsT=(
                                xnT_w[s // GC][k][
                                    :, (s % GC) * P : (s % GC + 1) * P
                                ]
                            ),
                            rhs=(wv[k][:, dc * 512 : (dc + 1) * 512]),
                            start=(k == 0),
                            stop=(k == KD - 1),
                        )
                nc.vector.tensor_add(vres[s][:, 0:D], ps, vb_sb)
                nc.vector.tensor_copy(vres[s][:, D : D + 1], onez_sb[:, 0:1])

            for m in range(KD):
                psK = pskg.tile([P, 2, 512], f32, tag="psK")
                psG = pskg.tile([P, 2, 512], f32, tag="psG")
                for k in range(KD):
                    wkt = wpool.tile([P, P], fmm, tag="wk")
                    wgt = wpool.tile([P, P], fmm, tag="wg")
                    nc.sync.dma_start(out=wkt, in_=wk_t[m, k])
                    nc.sync.dma_start(out=wgt, in_=wg_t[m, k])
                    for w, sc in enumerate(scs):
                        nc.tensor.matmul(
                            out=psK[:, w],
                            lhsT=(wkt),
                            rhs=(xnT_w[sc][k]),
                            start=(k == 0),
                            stop=(k == KD - 1),
                        )
                        nc.tensor.matmul(
                            out=psG[:, w],
                            lhsT=(wgt),
                            rhs=(xnT_w[sc][k]),
                            start=(k == 0),
                            stop=(k == KD - 1),
                        )
                for w, sc in enumerate(scs):
                    cols = slice(sc * 512, (sc + 1) * 512)
                    gg = epool.tile([P, 512], f32, tag="g")
                    nc.scalar.activation(
                        out=gg,
                        in_=psG[:, w],
                        func=Act.Sigmoid,
                        bias=bg_sb[:, m : m + 1],
                        scale=1.0,
                    )
                    kg = epool.tile([P, 512], f32, tag="kg")
                    nc.vector.scalar_tensor_tensor(
                        out=kg,
                        in0=psK[:, w],
                        scalar=bk_sb[:, m : m + 1],
                        in1=gg,
                        op0=Alu.add,
                        op1=Alu.mult,
                    )
                    m0 = epool.tile([P, 512], f32, tag="m0")
                    nc.vector.tensor_scalar(
                        out=m0, in0=kg, scalar1=0.0, scalar2=None, op0=Alu.min
                    )
                    e = epool.tile([P, 512], f32, tag="e")
                    nc.scalar.activation(out=e, in_=m0, func=Act.Exp)
                    nc.vector.scalar_tensor_tensor(
                        out=kT[m][:, cols],
                        in0=kg,
                        scalar=0.0,
                        in1=e,
                        op0=Alu.max,
                        op1=Alu.add,
                    )
                # one v slot between KG m-groups: its matmuls keep the PE
                # busy while psK/psG evictions drain
                v_slot(slots[m])
            psv.release()
            pskg.release()
            epool.release()
            wpool.release()

        wvpool.release()
        xnT_pool.release()

        # =========== phase ATTN: attnT[j] = kT_j.T @ qT, masked ==========
        # s-slot j is needed by local t-chunks i >= j%8; the first 128
        # t-cols of each eviction get the boundary mask (triangle for
        # j<8, all-ones/zeros by parity for j>=8), the rest copy.
        attn_pool = tc.alloc_tile_pool(name="attnT", bufs=1)
        attnT = []
        tstart = []
        for j in range(NT):
            t0 = (j % NL) * P
            tstart.append(t0)
            attnT.append(
                attn_pool.tile(
                    [P, NL * P - t0], fmm, tag=f"attnT{j}", name=f"attnT{j}"
                )
            )
        psa = tc.alloc_tile_pool(name="psA", bufs=3, space="PSUM")
        # interleave [0,8,1,9,...]: OUT chunk i needs slots {j: j%8<=i},
        # so pairing the two j%8==h slots lets OUT(i) start after 2(i+1)
        # evictions instead of waiting most of the phase
        for j in [h + r * NL for h in range(NL) for r in range(2)]:
            ntj = NL * P - tstart[j]
            ps = psa.tile([P, 1024], f32, tag="psA")
            for k in range(KD):
                for sub in range(0, ntj, 512):
                    w = min(512, ntj - sub)
                    nc.tensor.matmul(
                        out=ps[:, sub : sub + w],
                        lhsT=(kT[k][:, j * P : (j + 1) * P]),
                        rhs=(qT[k][:, tstart[j] + sub : tstart[j] + sub + w]),
                        start=(k == 0),
                        stop=(k == KD - 1),
                    )
            # masked eviction: first 128 cols get mask, rest plain copy
            msel = m0_sb if j < NL else m1_sb
            nc.vector.tensor_mul(attnT[j][:, 0:P], ps[:, 0:P], msel)
            if ntj > P:
                nc.scalar.copy(out=attnT[j][:, P:ntj], in_=ps[:, P:ntj])
        psa.release()
        kT_pool.release()
        qT_pool.release()

        # =========== phase OUT: out = (attnT.T @ v_aug), then /den =======
        fpool = tc.alloc_tile_pool(name="fin", bufs=3)
        pso = tc.alloc_tile_pool(name="psO", bufs=3, space="PSUM")
        psd = tc.alloc_tile_pool(name="psD", bufs=2, space="PSUM")
        for i in range(NL):
            js = [j for j in range(NT) if j % NL <= i]
            ps = pso.tile([P, D], f32, tag="psO")
            pd = psd.tile([P, 1], f32, tag="psDt")
            for idx, j in enumerate(js):
                acol = (i - j % NL) * P
                lhs = attnT[j][:, acol : acol + P]
                for s0, s1 in ((0, 512), (512, 1024)):
                    nc.tensor.matmul(
                        out=ps[:, s0:s1],
                        lhsT=(lhs),
                        rhs=(vres[j][:, s0:s1]),
                        start=(idx == 0),
                        stop=(idx == len(js) - 1),
                    )
                nc.tensor.matmul(
                    out=pd,
                    lhsT=(lhs),
                    rhs=(vres[j][:, D : D + 1]),
                    start=(idx == 0),
                    stop=(idx == len(js) - 1),
                )
            # finalize row-chunk i: out = num / (den + eps)
            di = fpool.tile([P, 1], f32, tag="di")
            nc.vector.tensor_scalar(
                out=di,
                in0=pd,
                scalar1=DEN_EPS,
                scalar2=None,
                op0=Alu.add,
            )
            dr = fpool.tile([P, 1], f32, tag="dr")
            nc.vector.reciprocal(out=dr, in_=di)
            otile = fpool.tile([P, D], fio, tag="otile")
            nc.vector.tensor_scalar_mul(out=otile, in0=ps, scalar1=dr)
            nc.sync.dma_start(out=out_d[i * P : (i + 1) * P, :], in_=otile)
        psd.release()
        pso.release()
        fpool.release()
        attn_pool.release()
        v_pool.release()
        dram.release()
        const.release()

    return nc


def _prep_consts(inputs, prec="f16"):
    """Fold ln into the projections, tile the weights, cast to fp16."""
    qkv_w = np.asarray(inputs["qkv_w"], dtype=np.float32)
    qkv_b = np.asarray(inputs["qkv_b"], dtype=np.float32)
    gate_w = np.asarray(inputs["gate_w"], dtype=np.float32)
    gate_b = np.asarray(inputs["gate_b"], dtype=np.float32)
    ln_g = np.asarray(inputs["ln_g"], dtype=np.float32)
    ln_b = np.asarray(inputs["ln_b"], dtype=np.float32)

    dt = np.float16 if prec == "f16" else np.float32

    w_eff = qkv_w * ln_g[:, None]
    b_eff = (qkv_b + ln_b @ qkv_w).astype(np.float32)
    wg_eff = gate_w * ln_g[:, None]
    bg_eff = (gate_b + ln_b @ gate_w).astype(np.float32)

    # w[din, dout] -> tiles[m, k] = w[k*P:(k+1)*P, m*P:(m+1)*P]
    def tiles_mk(w):
        return np.ascontiguousarray(
            w.reshape(KD, P, KD, P).transpose(2, 0, 1, 3).astype(dt)
        )

    return {
        "wq_t": tiles_mk(w_eff[:, 0:D]),
        "wk_t": tiles_mk(w_eff[:, D : 2 * D]),
        "wg_t": tiles_mk(wg_eff),
        "wv_t": np.ascontiguousarray(
            w_eff[:, 2 * D : 3 * D].reshape(KD, P, D).astype(dt)
        ),
        "bqkv": b_eff,
        "bg": bg_eff,
    }


def _host_globals(inputs, prec="f16"):
    """Concatenated per-core runtime inputs (axis 0 = core-major).

    Each core gets only its own-parity chunks of its batch (the pair
    AllGather reconstitutes the full batch on device in [even | odd]
    slot order).  Boundary masks for that slot order: parity 0 sees
    (triangle, zeros), parity 1 sees (ones, triangle).
    """
    dt = np.float16 if prec == "f16" else np.float32
    x = np.asarray(inputs["x"])
    # [B, NL, par, P, D] -> [B, par, NL, P, D] view, one cast pass to dt
    xg = np.ascontiguousarray(
        x.reshape(B, NL, 2, P, D).transpose(0, 2, 1, 3, 4), dtype=dt
    )

    if "masks" not in _CACHE:
        tri = np.triu(np.ones((P, P), dtype=np.float32))
        mg = np.empty((B, 2, 2, P, P), dtype=np.float32)
        mg[:, 0, 0] = tri
        mg[:, 0, 1] = 0.0
        mg[:, 1, 0] = 1.0
        mg[:, 1, 1] = tri
        _CACHE["masks"] = mg.reshape(N_CORES * 2, P, P)
    return {
        "x": xg.reshape(N_CORES * NL * P, D),
        "masks": _CACHE["masks"],
    }


def _host_core_inputs(inputs, prec="f16"):
    """Per-core input dicts (for CoreSim / run_bass_kernel_spmd)."""
    g = _host_globals(inputs, prec)
    x = g["x"].reshape(N_CORES, NL * P, D)
    m = g["masks"].reshape(N_CORES, 2, P, P)
    return [{"x": x[c], "masks": m[c]} for c in range(N_CORES)]


def _host_assemble(out_flat):
    """[N_CORES*NL*P, D] core-major chunks -> [B, T, D] float32."""
    o = np.asarray(out_flat).reshape(B, 2, NL, P, D)
    # [B, par, i, P, D] -> [B, i, par, P, D]: chunk g = 2i+par; single
    # strided cast pass to float32
    full = np.ascontiguousarray(
        o.transpose(0, 2, 1, 3, 4), dtype=np.float32
    )
    return full.reshape(B, T, D)


def _build_exec(nc, n_cores=N_CORES):
    """Compile nc once into a cached sharded jit callable.

    Mirrors bass2jax.run_bass_via_pjrt's multi-core path, but (a) the
    jitted function is built once and reused (run_bass_via_pjrt re-traces
    and re-lowers per call, which also re-runs the Const->ExternalInput
    lowering mutation and breaks on the second call), and (b) the donated
    zero output buffers are created on-device instead of being uploaded.
    """
    import jax
    import jax.numpy as jnp
    from jax.sharding import Mesh, NamedSharding, PartitionSpec
    from jax.experimental.shard_map import shard_map
    from concourse import bass2jax, mybir

    bass2jax.install_neuronx_cc_hook()
    assert nc.dbg_addr is None
    partition_name = (
        nc.partition_id_tensor.name if nc.partition_id_tensor else None
    )

    in_names, out_names, out_avals = [], [], []
    for alloc in nc.m.functions[0].allocations:
        if not isinstance(alloc, mybir.MemoryLocationSet):
            continue
        name = alloc.memorylocations[0].name
        if alloc.kind == "ExternalInput":
            if name != partition_name:
                in_names.append(name)
        elif alloc.kind == "ExternalOutput":
            out_names.append(name)
            out_avals.append(
                jax.core.ShapedArray(
                    tuple(alloc.tensor_shape), mybir.dt.np(alloc.dtype)
                )
            )
    n_params = len(in_names)
    n_outs = len(out_names)
    all_in = tuple(in_names) + tuple(out_names)
    if partition_name is not None:
        all_in = all_in + (partition_name,)

    def _body(*args):
        operands = list(args)
        if partition_name is not None:
            operands.append(bass2jax.partition_id_tensor())
        outs = bass2jax._bass_exec_p.bind(
            *operands,
            out_avals=tuple(out_avals),
            in_names=all_in,
            out_names=tuple(out_names),
            lowering_input_output_aliases=(),
            sim_require_finite=True,
            sim_require_nnan=True,
            nc=nc,
        )
        return tuple(outs)

    devices = jax.devices()[:n_cores]
    mesh = Mesh(np.asarray(devices), ("core",))
    spec = PartitionSpec("core")
    donate = tuple(range(n_params, n_params + n_outs))
    sharded = jax.jit(
        shard_map(
            _body,
            mesh=mesh,
            in_specs=(spec,) * (n_params + n_outs),
            out_specs=(spec,) * n_outs,
            check_rep=False,
        ),
        donate_argnums=donate,
        keep_unused=True,
    )
    sh = NamedSharding(mesh, spec)
    gshapes = [
        (n_cores * av.shape[0], *av.shape[1:]) for av in out_avals
    ]
    gdtypes = [av.dtype for av in out_avals]
    zeros_fn = jax.jit(
        lambda: tuple(jnp.zeros(s, d) for s, d in zip(gshapes, gdtypes)),
        out_shardings=tuple(sh for _ in gshapes),
    )

    dev_cache = {}

    def run(global_inputs):
        args = []
        for name in in_names:
            if name == "masks":
                # call-invariant input: keep it device-resident
                if name not in dev_cache:
                    dev_cache[name] = jax.device_put(global_inputs[name], sh)
                args.append(dev_cache[name])
            else:
                args.append(global_inputs[name])
        zs = zeros_fn()
        outs = sharded(*args, *zs)
        return dict(zip(out_names, outs))

    return run


def _weights_key(inputs):
    import hashlib

    h = hashlib.md5()
    for k in ("qkv_w", "qkv_b", "gate_w", "gate_b", "ln_g", "ln_b"):
        a = np.ascontiguousarray(inputs[k])
        h.update(k.encode())
        h.update(str(a.shape).encode())
        h.update(str(a.dtype).encode())
        h.update(a.tobytes()[::257])
    return h.hexdigest()


def kernel(**inputs):
    prec = "f16"
    key = ("exec", prec, _weights_key(inputs))
    if key not in _CACHE:
        consts = _prep_consts(inputs, prec)
        nc = build_program(prec=prec, consts=consts)
        _CACHE[key] = _build_exec(nc)
    run = _CACHE[key]
    outs = run(_host_globals(inputs, prec))
    return _host_assemble(outs["out"])


# revision 67
# speedup vs baseline: 1.1806x; 1.1806x over previous
"""Trainium2 Bass kernel for CausalGatedD2Attention.

Math (per batch b):
  xn   = LayerNorm(x) * ln_g + ln_b            [T, D]
  qkv  = xn @ qkv_w + qkv_b                     -> q, k, v  [T, D] each
  gate = sigmoid(xn @ gate_w + gate_b)
  k    = elu(k * gate) + 1 ;  q = elu(q) + 1
  attn = tril(q @ k^T)                          [T, T]
  out  = (attn @ v) / (rowsum(attn) + eps)      [T, D]

Sharding: 4 batches x 2 cores.  Within a pair, core parity par in {0,1}
owns the even/odd 128-row t-chunks of its batch (balances the causal
triangle).  Each core receives its batch CHUNK-PERMUTED: its own-parity
chunks first (slots 0..7 = local t-chunks), the complementary chunks
last (slots 8..15).  With that layout one uniform program serves all
cores:
  - q rows are always slots 0..7 (the first half),
  - k/v cover all 16 slots,
  - causality becomes slot-triangular: t-chunk i attends s-slots
    {j : j%8 <= i}; only the two boundary diagonals depend on parity
    and those are handled by two host-provided [128,128] mask tiles
    (triangle for j<8, all-ones/all-zeros for j>=8).

Weights / biases are identical on every call, so they are baked into
the program as Const (inline) tensors: they ship inside the compiled
executable once instead of being re-uploaded per execution.  Only the
fp16 activations x (and two tiny mask tiles) move host->device per
call, and only the fp16 output moves back.  Compute accumulates in
fp32 PSUM.  ln_g / ln_b are folded into the projection weights on the
host, so the device LN is just (x - mean) * rsqrt(var + eps).

The denominator comes for free: v gets an appended ones-column, so
attn @ v_aug yields [num | den] in one accumulation.
"""

import sys

sys.path.insert(0, "/opt/trn_rl_repo")

import numpy as np

B, T, D = 4, 2048, 1024
P = 128
KD = D // P          # 8 contraction chunks
NT = T // P          # 16 chunk slots
NL = NT // 2         # 8 local t-chunks per core
LN_EPS = 1e-5
DEN_EPS = 1e-6
N_CORES = 8

_CACHE = {}


def _patched_tc(tile_mod):
    import bass_rust as _br
    from concourse.vector_clock import ScopedClock

    class TC(tile_mod.TileContext):
        """TileContext whose final drain splits sem waits one per
        instruction (walrus CoreV3 allows a single wait on Drain)."""

        def _spread_waits(self):
            # walrus allows at most 2 sem waits on engine instructions and
            # only 1 on CTRL-class ones (Drain/NoOp); Tile's scheduler can
            # emit more.  Move excess waits onto same-engine nops placed
            # immediately before the over-limit instruction.
            nc = self.nc
            for fnbb in nc.m.functions[0].blocks:
                insts = list(fnbb.instructions)
                out = []
                for inst in insts:
                    si = inst.sync_info
                    waits = list(si.on_wait) if si is not None else []
                    limit = 1
                    if len(waits) > limit:
                        excess = waits[limit:]
                        si.on_wait = waits[:limit]
                        inst.sync_info = si
                        for w in excess:
                            nop = nc.engines[inst.engine].nop(
                                nofuse=True, hint="wait_spread"
                            )
                            nop.ins.sync_info = _br.SyncInfo(
                                on_wait=[w], on_update=[]
                            )
                            # remove from wherever it was appended
                            for b2 in nc.m.functions[0].blocks:
                                cur = list(b2.instructions)
                                if cur and cur[-1] is nop.ins:
                                    b2.instructions = cur[:-1]
                                    break
                            out.append(nop.ins)
                    out.append(inst)
                fnbb.instructions = out

        def _drain_and_barrier(self, tick_clock, wait_clock):
            self._spread_waits()
            drain_inst = self.nc.sync.drain()
            wait_clock.add_sem_waits(
                drain_inst.ins, ScopedClock({None: tick_clock.global_clock})
            )
            si = drain_inst.ins.sync_info
            waits = list(si.on_wait)
            if len(waits) > 1:
                si.on_wait = waits[:1]
                drain_inst.ins.sync_info = si
                for i in range(1, len(waits)):
                    nop = self.nc.sync.nop(nofuse=True, hint="drain_extra_waits")
                    nop.ins.sync_info = _br.SyncInfo(
                        on_wait=waits[i : i + 1], on_update=[]
                    )
            self.nc.all_engine_barrier()
            assert self.sems is not None
            popped = self.nc._tile_sem_poison_stack.pop()
            assert popped is self._sem_poison
            self.nc.clear_and_free_semaphores(list(self.sems.allocated().values()))
            self.nc.all_engine_barrier()

    return TC


def build_program(prec="f16", consts=None):
    import concourse.bass as bass
    import concourse.tile as tile
    from concourse import mybir
    from concourse.masks import make_identity

    TC = _patched_tc(tile)
    f32 = mybir.dt.float32
    Act = mybir.ActivationFunctionType
    Alu = mybir.AluOpType

    if prec == "f16":
        fio = mybir.dt.float16   # host<->device payload dtype
        fmm = mybir.dt.float16   # matmul operand dtype
    else:
        fio = f32
        fmm = f32

    nc = bass.Bass(num_devices=N_CORES)
    x_in = nc.declare_dram_parameter("x", [NL * P, D], fio, isOutput=False)
    masks_in = nc.declare_dram_parameter("masks", [2, P, P], f32, isOutput=False)
    out_d = nc.declare_dram_parameter("out", [NL * P, D], fio, isOutput=True)
    # weights ride inside the executable as load-time constants
    wq_t = nc.inline_tensor(consts["wq_t"], name="wq_t")
    wk_t = nc.inline_tensor(consts["wk_t"], name="wk_t")
    wg_t = nc.inline_tensor(consts["wg_t"], name="wg_t")
    wv_t = nc.inline_tensor(consts["wv_t"], name="wv_t")
    bqkv = nc.inline_tensor(consts["bqkv"], name="bqkv")
    bg_in = nc.inline_tensor(consts["bg"], name="bg")

    with TC(nc) as tc:
        const = tc.alloc_tile_pool(name="const", bufs=1)
        ident = const.tile([P, P], fmm, tag="ident")
        make_identity(nc, ident)
        # =========== phase CC: pair AllGather of x =======================
        # Each core uploads only its own-parity 128-row chunks (slots in
        # ascending global order).  The pair AllGather yields the full
        # batch in [even-chunks | odd-chunks] slot order on both cores.
        # Triggered first (in two pipelined groups of GC chunks) so the
        # wire time hides under the local-q LN + q-projection below,
        # which only need the local x; the full-batch phases then
        # consume gathered groups as they land.
        GC = NL // 2  # chunks per gather group per rank
        dram = tc.alloc_tile_pool(name="ccdram", bufs=1, space="DRAM")
        xtloc = [
            dram.tile([KD, P, GC * P], fmm, tag=f"xtloc{g}", name=f"xtloc{g}")
            for g in range(2)
        ]
        xtall = [
            dram.tile([2, KD, P, GC * P], fmm, tag=f"xtall{g}", name=f"xtall{g}")
            for g in range(2)
        ]
        rgroups = [[2 * p, 2 * p + 1] for p in range(N_CORES // 2)]

        wkg_pool = tc.alloc_tile_pool(name="wkgres", bufs=1)
        wk_sb = [
            wkg_pool.tile([P, KD * P], fmm, tag=f"wkr{m}", name=f"wkr{m}")
            for m in range(KD)
        ]
        wg_sb = [
            wkg_pool.tile([P, KD * P], fmm, tag=f"wgr{m}", name=f"wgr{m}")
            for m in range(KD)
        ]

        # biases: [P, KD] with column m = bias[m*128:(m+1)*128]
        bq_sb = const.tile([P, KD], f32, tag="bq")
        bk_sb = const.tile([P, KD], f32, tag="bk")
        bg_sb = const.tile([P, KD], f32, tag="bgs")
        # transposed bias loads are ~1k tiny descriptors each; keep them
        # off the sync DMA queue so the gather's bounce copy goes first
        b3 = bqkv.rearrange("(s m p) -> s m p", s=3, m=KD, p=P)
        nc.scalar.dma_start(out=bq_sb, in_=b3[0].rearrange("m p -> p m"))
        nc.scalar.dma_start(out=bk_sb, in_=b3[1].rearrange("m p -> p m"))
        nc.scalar.dma_start(
            out=bg_sb, in_=bg_in.rearrange("(m p) -> p m", m=KD, p=P)
        )
        ln_eps = const.tile([P, 1], f32, tag="lneps")
        nc.vector.memset(ln_eps, LN_EPS)
        onez_sb = const.tile([P, 2], f32, tag="onez")
        nc.vector.memset(onez_sb[:, 0:1], 1.0)
        nc.vector.memset(onez_sb[:, 1:2], 0.0)
        m0_sb = const.tile([P, P], f32, tag="m0sb")
        m1_sb = const.tile([P, P], f32, tag="m1sb")
        nc.scalar.dma_start(out=m0_sb, in_=masks_in[0])
        nc.scalar.dma_start(out=m1_sb, in_=masks_in[1])

        # ========= phase XQ: local layernorm + transpose -> xqnT =========
        # (for the q side; local rows only, independent of the gather)
        def ln_chunk(src_ap, dstT, c, xpool, spool, pspool):
            xt = xpool.tile([P, D], fio, tag="xt")
            nc.sync.dma_start(out=xt, in_=src_ap)
            stats = spool.tile([P, 2, 6], f32, tag="stats")
            xr = xt.rearrange("p (n f) -> p n f", n=2)
            for sg in range(2):
                nc.vector.bn_stats(out=stats[:, sg], in_=xr[:, sg])
            mv = spool.tile([P, 2], f32, tag="mv")
            nc.vector.bn_aggr(out=mv, in_=stats)
            rstd = spool.tile([P, 1], f32, tag="rstd")
            nc.scalar.activation(
                out=rstd, in_=mv[:, 1:2], func=Act.Sqrt, bias=ln_eps, scale=1.0
            )
            rstd2 = spool.tile([P, 1], f32, tag="rstd2")
            nc.vector.reciprocal(out=rstd2, in_=rstd)
            nmr = spool.tile([P, 1], f32, tag="nmr")
            nc.vector.tensor_scalar(
                out=nmr,
                in0=mv[:, 0:1],
                scalar1=rstd2,
                scalar2=-1.0,
                op0=Alu.mult,
                op1=Alu.mult,
            )
            xn = xpool.tile([P, D], fmm, tag="xn")
            nc.scalar.activation(
                out=xn, in_=xt, func=Act.Identity, bias=nmr, scale=rstd2
            )
            for k in range(KD):
                ps = pspool.tile([P, P], fmm, tag="psT")
                nc.tensor.transpose(
                    out=ps, in_=xn[:, k * P : (k + 1) * P], identity=ident
                )
                if k % 2 == 0:
                    nc.vector.tensor_copy(dstT[k][:, c * P : (c + 1) * P], ps)
                else:
                    nc.scalar.copy(out=dstT[k][:, c * P : (c + 1) * P], in_=ps)

        # xqnT is split per gather group so each group's pack (and its
        # AllGather of LN'd+transposed columns) can fire as soon as that
        # group's chunks are done; the partner's xnT arrives ready, so no
        # full-batch LN/transpose phase exists at all.
        xqnT_pool = tc.alloc_tile_pool(name="xqnT", bufs=1)
        xqnT_g = [
            [
                xqnT_pool.tile(
                    [P, GC * P], fmm, tag=f"xqnT{g}_{k}", name=f"xqnT{g}_{k}"
                )
                for k in range(KD)
            ]
            for g in range(2)
        ]
        xpool = tc.alloc_tile_pool(name="xqwork", bufs=3)
        spool = tc.alloc_tile_pool(name="xqstat", bufs=4)
        pspool = tc.alloc_tile_pool(name="psTq", bufs=4, space="PSUM")
        for g in range(2):
            for i in range(GC):
                c = g * GC + i
                ln_chunk(
                    x_in[c * P : (c + 1) * P, :],
                    xqnT_g[g],
                    i,
                    xpool,
                    spool,
                    pspool,
                )
            for k in range(KD):
                eng = nc.sync if k % 2 == 0 else nc.scalar
                eng.dma_start(out=xtloc[g][k], in_=xqnT_g[g][k])
            nc.gpsimd.collective_compute(
                "AllGather",
                mybir.AluOpType.bypass,
                replica_groups=rgroups,
                ins=[xtloc[g].opt()],
                outs=[xtall[g].opt()],
            )
        pspool.release()
        spool.release()
        xpool.release()

        # v tiles live SBUF-resident until the OUT phase; allocate their
        # pool first so right-side pools release in stack order
        # (kT, qT, vres).
        v_pool = tc.alloc_tile_pool(name="vres", bufs=1, side="right")
        vres = [
            v_pool.tile([P, D + 1], fmm, tag=f"v{s}", name=f"v{s}")
            for s in range(NT)
        ]

        # k/gate weight DMAs (tiles allocated up top, before xqnT, for
        # pool-stack order): emitted here so the transfers fill the
        # gather-wait window; KG then has no per-(m,k) load latency.
        for m in range(KD):
            nc.sync.dma_start(
                out=wk_sb[m].rearrange("p (k q) -> p k q", k=KD),
                in_=wk_t[m].rearrange("k p q -> p k q"),
            )
            nc.scalar.dma_start(
                out=wg_sb[m].rearrange("p (k q) -> p k q", k=KD),
                in_=wg_t[m].rearrange("k p q -> p k q"),
            )

        # =========== phase QP: q projection -> qT (elu+1) ================
        # (also independent of the gather; elu min runs on DVE, not
        # gpsimd, so it cannot queue behind the collective)
        qT_pool = tc.alloc_tile_pool(name="qT", bufs=1, side="right")
        qT = [
            qT_pool.tile([P, NL * P], fmm, tag=f"qT{m}", name=f"qT{m}")
            for m in range(KD)
        ]
        wpool = tc.alloc_tile_pool(name="wq", bufs=4)
        epool = tc.alloc_tile_pool(name="qev", bufs=3)
        psq = tc.alloc_tile_pool(name="psQ", bufs=4, space="PSUM")
        for m in range(KD):
            ps = psq.tile([P, NL * P], f32, tag="psQ")
            for k in range(KD):
                wqt = wpool.tile([P, P], fmm, tag="wqt")
                nc.sync.dma_start(out=wqt, in_=wq_t[m, k])
                for sc in range(2):
                    nc.tensor.matmul(
                        out=ps[:, sc * 512 : (sc + 1) * 512],
                        lhsT=(wqt),
                        rhs=(xqnT_g[sc][k]),
                        start=(k == 0),
                        stop=(k == KD - 1),
                    )
            for sc in range(2):
                cols = slice(sc * 512, (sc + 1) * 512)
                qx = epool.tile([P, 512], f32, tag="qx")
                nc.scalar.activation(
                    out=qx,
                    in_=ps[:, cols],
                    func=Act.Identity,
                    bias=bq_sb[:, m : m + 1],
                    scale=1.0,
                )
                m0 = epool.tile([P, 512], f32, tag="qm0")
                nc.vector.tensor_scalar(
                    out=m0, in0=qx, scalar1=0.0, scalar2=None, op0=Alu.min
                )
                e = epool.tile([P, 512], f32, tag="qe")
                nc.scalar.activation(out=e, in_=m0, func=Act.Exp)
                nc.vector.scalar_tensor_tensor(
                    out=qT[m][:, cols],
                    in0=qx,
                    scalar=0.0,
                    in1=e,
                    op0=Alu.max,
                    op1=Alu.add,
                )
        psq.release()
        epool.release()
        wpool.release()
        xqnT_pool.release()

        # ====== phases XF/KG/V: consume gathered x group by group ========
        # Group g lands slots {g*GC..g*GC+GC-1} (even rank) and
        # {NL+g*GC..} (odd rank): LN+transpose them into xnT, then run
        # the k/gate projections for those 512-wide column windows and
        # the v projections for those slots, while the next gather
        # group is still in flight.
        # xnT as 4 window tiles per k (window w = slots {4w..4w+3}):
        # group g's unpack fills windows {g, 2+g} straight from the
        # gathered, already-LN'd-and-transposed columns
        xnT_pool = tc.alloc_tile_pool(name="xnT", bufs=1)
        xnT_w = [
            [
                xnT_pool.tile(
                    [P, GC * P], fmm, tag=f"xnT{w}_{k}", name=f"xnT{w}_{k}"
                )
                for k in range(KD)
            ]
            for w in range(4)
        ]
        wvpool = tc.alloc_tile_pool(name="wv", bufs=1)
        vb_sb = wvpool.tile([P, D], f32, tag="vb", name="vb_sb")
        vslice = b3[2].rearrange("m p -> (m p)")
        vb_bcast = bass.AP(
            tensor=vslice.tensor, offset=vslice.offset, ap=[[0, P], *vslice.ap]
        )
        nc.sync.dma_start(out=vb_sb, in_=vb_bcast)
        wv = []
        for k in range(KD):
            t = wvpool.tile([P, D], fmm, tag=f"wv{k}", name=f"wv{k}")
            nc.sync.dma_start(out=t, in_=wv_t[k])
            wv.append(t)
        kT_pool = tc.alloc_tile_pool(name="kT", bufs=1, side="right")
        kT = [
            kT_pool.tile([P, T], fmm, tag=f"kT{m}", name=f"kT{m}")
            for m in range(KD)
        ]
        for g in range(2):
            slots = [r * NL + g * GC + i for r in range(2) for i in range(GC)]
            # --- unpack_g: gathered xnT columns -> SBUF window tiles ---
            for k in range(KD):
                eng = nc.sync if k % 2 == 0 else nc.scalar
                eng.dma_start(out=xnT_w[g][k], in_=xtall[g][0, k])
                eng.dma_start(out=xnT_w[2 + g][k], in_=xtall[g][1, k])
            # --- KG_g: k/gate projections for this group's columns ---
            wpool = tc.alloc_tile_pool(name="wkg", bufs=8)
            epool = tc.alloc_tile_pool(name="kgev", bufs=2)
            pskg = tc.alloc_tile_pool(name="psKG", bufs=1, space="PSUM")
            psv = tc.alloc_tile_pool(name="psV", bufs=2, space="PSUM")
            scs = [g, 2 + g]  # 512-col windows of this group

            def v_slot(s):
                ps = psv.tile([P, D], f32, tag="psV", name="ps")
                for k in range(KD):
                    for dc in range(2):
                        nc.tensor.matmul(
                            out=ps[:, dc * 512 : (dc + 1) * 512],
                            lhsT=(
                                xnT_w[s // GC][k][
                                    :, (s % GC) * P : (s % GC + 1) * P
                                ]
                            ),
                            rhs=(wv[k][:, dc * 512 : (dc + 1) * 512]),
                            start=(k == 0),
                            stop=(k == KD - 1),
                        )
                nc.vector.tensor_add(vres[s][:, 0:D], ps, vb_sb)
                nc.vector.tensor_copy(vres[s][:, D : D + 1], onez_sb[:, 0:1])

            for m in range(KD):
                psK = pskg.tile([P, 2, 512], f32, tag="psK")
                psG = pskg.tile([P, 2, 512], f32, tag="psG")
                for k in range(KD):
                    kcols = slice(k * P, (k + 1) * P)
                    for w, sc in enumerate(scs):
                        nc.tensor.matmul(
                            out=psK[:, w],
                            lhsT=(wk_sb[m][:, kcols]),
                            rhs=(xnT_w[sc][k]),
                            start=(k == 0),
                            stop=(k == KD - 1),
                        )
                        nc.tensor.matmul(
                            out=psG[:, w],
                            lhsT=(wg_sb[m][:, kcols]),
                            rhs=(xnT_w[sc][k]),
                            start=(k == 0),
                            stop=(k == KD - 1),
                        )
                for w, sc in enumerate(scs):
                    cols = slice(sc * 512, (sc + 1) * 512)
                    gg = epool.tile([P, 512], f32, tag="g")
                    nc.scalar.activation(
                        out=gg,
                        in_=psG[:, w],
                        func=Act.Sigmoid,
                        bias=bg_sb[:, m : m + 1],
                        scale=1.0,
                    )
                    kg = epool.tile([P, 512], f32, tag="kg")
                    nc.vector.scalar_tensor_tensor(
                        out=kg,
                        in0=psK[:, w],
                        scalar=bk_sb[:, m : m + 1],
                        in1=gg,
                        op0=Alu.add,
                        op1=Alu.mult,
                    )
                    m0 = epool.tile([P, 512], f32, tag="m0")
                    nc.vector.tensor_scalar(
                        out=m0, in0=kg, scalar1=0.0, scalar2=None, op0=Alu.min
                    )
                    e = epool.tile([P, 512], f32, tag="e")
                    nc.scalar.activation(out=e, in_=m0, func=Act.Exp)
                    nc.vector.scalar_tensor_tensor(
                        out=kT[m][:, cols],
                        in0=kg,
                        scalar=0.0,
                        in1=e,
                        op0=Alu.max,
                        op1=Alu.add,
                    )
                # one v slot between KG m-groups: its matmuls keep the PE
                # busy while psK/psG evictions drain
                v_slot(slots[m])
            psv.release()
            pskg.release()
            epool.release()
            wpool.release()

        wvpool.release()
        xnT_pool.release()

        # =========== phase ATTN: attnT[j] = kT_j.T @ qT, masked ==========
        # s-slot j is needed by local t-chunks i >= j%8; the first 128
        # t-cols of each eviction get the boundary mask (triangle for
        # j<8, all-ones/zeros by parity for j>=8), the rest copy.
        attn_pool = tc.alloc_tile_pool(name="attnT", bufs=1)
        attnT = []
        tstart = []
        for j in range(NT):
            t0 = (j % NL) * P
            tstart.append(t0)
            attnT.append(
                attn_pool.tile(
                    [P, NL * P - t0], fmm, tag=f"attnT{j}", name=f"attnT{j}"
                )
            )
        psa = tc.alloc_tile_pool(name="psA", bufs=3, space="PSUM")
        # interleave [0,8,1,9,...]: OUT chunk i needs slots {j: j%8<=i},
        # so pairing the two j%8==h slots lets OUT(i) start after 2(i+1)
        # evictions instead of waiting most of the phase
        for j in [h + r * NL for h in range(NL) for r in range(2)]:
            ntj = NL * P - tstart[j]
            ps = psa.tile([P, 1024], f32, tag="psA")
            for k in range(KD):
                for sub in range(0, ntj, 512):
                    w = min(512, ntj - sub)
                    nc.tensor.matmul(
                        out=ps[:, sub : sub + w],
                        lhsT=(kT[k][:, j * P : (j + 1) * P]),
                        rhs=(qT[k][:, tstart[j] + sub : tstart[j] + sub + w]),
                        start=(k == 0),
                        stop=(k == KD - 1),
                    )
            # masked eviction: first 128 cols get mask, rest plain copy
            msel = m0_sb if j < NL else m1_sb
            nc.vector.tensor_mul(attnT[j][:, 0:P], ps[:, 0:P], msel)
            if ntj > P:
                nc.scalar.copy(out=attnT[j][:, P:ntj], in_=ps[:, P:ntj])
        psa.release()
        kT_pool.release()
        qT_pool.release()

        # =========== phase OUT: out = (attnT.T @ v_aug), then /den =======
        fpool = tc.alloc_tile_pool(name="fin", bufs=3)
        pso = tc.alloc_tile_pool(name="psO", bufs=3, space="PSUM")
        psd = tc.alloc_tile_pool(name="psD", bufs=2, space="PSUM")
        for i in range(NL):
            js = [j for j in range(NT) if j % NL <= i]
            ps = pso.tile([P, D], f32, tag="psO")
            pd = psd.tile([P, 1], f32, tag="psDt")
            for idx, j in enumerate(js):
                acol = (i - j % NL) * P
                lhs = attnT[j][:, acol : acol + P]
                for s0, s1 in ((0, 512), (512, 1024)):
                    nc.tensor.matmul(
                        out=ps[:, s0:s1],
                        lhsT=(lhs),
                        rhs=(vres[j][:, s0:s1]),
                        start=(idx == 0),
                        stop=(idx == len(js) - 1),
                    )
                nc.tensor.matmul(
                    out=pd,
                    lhsT=(lhs),
                    rhs=(vres[j][:, D : D + 1]),
                    start=(idx == 0),
                    stop=(idx == len(js) - 1),
                )
            # finalize row-chunk i: out = num / (den + eps)
            di = fpool.tile([P, 1], f32, tag="di")
            nc.vector.tensor_scalar(
                out=di,
                in0=pd,
                scalar1=DEN_EPS,
                scalar2=None,
                op0=Alu.add,
            )
            dr = fpool.tile([P, 1], f32, tag="dr")
            nc.vector.reciprocal(out=dr, in_=di)
            otile = fpool.tile([P, D], fio, tag="otile")
            nc.vector.tensor_scalar_mul(out=otile, in0=ps, scalar1=dr)
            nc.sync.dma_start(out=out_d[i * P : (i + 1) * P, :], in_=otile)
        psd.release()
        pso.release()
        fpool.release()
        attn_pool.release()
        wkg_pool.release()
        v_pool.release()
        dram.release()
        const.release()

    return nc


def _prep_consts(inputs, prec="f16"):
    """Fold ln into the projections, tile the weights, cast to fp16."""
    qkv_w = np.asarray(inputs["qkv_w"], dtype=np.float32)
    qkv_b = np.asarray(inputs["qkv_b"], dtype=np.float32)
    gate_w = np.asarray(inputs["gate_w"], dtype=np.float32)
    gate_b = np.asarray(inputs["gate_b"], dtype=np.float32)
    ln_g = np.asarray(inputs["ln_g"], dtype=np.float32)
    ln_b = np.asarray(inputs["ln_b"], dtype=np.float32)

    dt = np.float16 if prec == "f16" else np.float32

    w_eff = qkv_w * ln_g[:, None]
    b_eff = (qkv_b + ln_b @ qkv_w).astype(np.float32)
    wg_eff = gate_w * ln_g[:, None]
    bg_eff = (gate_b + ln_b @ gate_w).astype(np.float32)

    # w[din, dout] -> tiles[m, k] = w[k*P:(k+1)*P, m*P:(m+1)*P]
    def tiles_mk(w):
        return np.ascontiguousarray(
            w.reshape(KD, P, KD, P).transpose(2, 0, 1, 3).astype(dt)
        )

    return {
        "wq_t": tiles_mk(w_eff[:, 0:D]),
        "wk_t": tiles_mk(w_eff[:, D : 2 * D]),
        "wg_t": tiles_mk(wg_eff),
        "wv_t": np.ascontiguousarray(
            w_eff[:, 2 * D : 3 * D].reshape(KD, P, D).astype(dt)
        ),
        "bqkv": b_eff,
        "bg": bg_eff,
    }


def _host_globals(inputs, prec="f16"):
    """Concatenated per-core runtime inputs (axis 0 = core-major).

    Each core gets only its own-parity chunks of its batch (the pair
    AllGather reconstitutes the full batch on device in [even | odd]
    slot order).  Boundary masks for that slot order: parity 0 sees
    (triangle, zeros), parity 1 sees (ones, triangle).
    """
    dt = np.float16 if prec == "f16" else np.float32
    x = np.asarray(inputs["x"])
    # [B, NL, par, P, D] -> [B, par, NL, P, D] view, one cast pass to dt
    xg = np.ascontiguousarray(
        x.reshape(B, NL, 2, P, D).transpose(0, 2, 1, 3, 4), dtype=dt
    )

    if "masks" not in _CACHE:
        tri = np.triu(np.ones((P, P), dtype=np.float32))
        mg = np.empty((B, 2, 2, P, P), dtype=np.float32)
        mg[:, 0, 0] = tri
        mg[:, 0, 1] = 0.0
        mg[:, 1, 0] = 1.0
        mg[:, 1, 1] = tri
        _CACHE["masks"] = mg.reshape(N_CORES * 2, P, P)
    return {
        "x": xg.reshape(N_CORES * NL * P, D),
        "masks": _CACHE["masks"],
    }


def _host_core_inputs(inputs, prec="f16"):
    """Per-core input dicts (for CoreSim / run_bass_kernel_spmd)."""
    g = _host_globals(inputs, prec)
    x = g["x"].reshape(N_CORES, NL * P, D)
    m = g["masks"].reshape(N_CORES, 2, P, P)
    return [{"x": x[c], "masks": m[c]} for c in range(N_CORES)]


def _host_assemble(out_flat):
    """[N_CORES*NL*P, D] core-major chunks -> [B, T, D] float32."""
    o = np.asarray(out_flat).reshape(B, 2, NL, P, D)
    # [B, par, i, P, D] -> [B, i, par, P, D]: chunk g = 2i+par; single
    # strided cast pass to float32
    full = np.ascontiguousarray(
        o.transpose(0, 2, 1, 3, 4), dtype=np.float32
    )
    return full.reshape(B, T, D)


def _build_exec(nc, n_cores=N_CORES):
    """Compile nc once into a cached sharded jit callable.

    Mirrors bass2jax.run_bass_via_pjrt's multi-core path, but (a) the
    jitted function is built once and reused (run_bass_via_pjrt re-traces
    and re-lowers per call, which also re-runs the Const->ExternalInput
    lowering mutation and breaks on the second call), and (b) the donated
    zero output buffers are created on-device instead of being uploaded.
    """
    import jax
    import jax.numpy as jnp
    from jax.sharding import Mesh, NamedSharding, PartitionSpec
    from jax.experimental.shard_map import shard_map
    from concourse import bass2jax, mybir

    bass2jax.install_neuronx_cc_hook()
    assert nc.dbg_addr is None
    partition_name = (
        nc.partition_id_tensor.name if nc.partition_id_tensor else None
    )

    in_names, out_names, out_avals = [], [], []
    for alloc in nc.m.functions[0].allocations:
        if not isinstance(alloc, mybir.MemoryLocationSet):
            continue
        name = alloc.memorylocations[0].name
        if alloc.kind == "ExternalInput":
            if name != partition_name:
                in_names.append(name)
        elif alloc.kind == "ExternalOutput":
            out_names.append(name)
            out_avals.append(
                jax.core.ShapedArray(
                    tuple(alloc.tensor_shape), mybir.dt.np(alloc.dtype)
                )
            )
    n_params = len(in_names)
    n_outs = len(out_names)
    all_in = tuple(in_names) + tuple(out_names)
    if partition_name is not None:
        all_in = all_in + (partition_name,)

    def _body(*args):
        operands = list(args)
        if partition_name is not None:
            operands.append(bass2jax.partition_id_tensor())
        outs = bass2jax._bass_exec_p.bind(
            *operands,
            out_avals=tuple(out_avals),
            in_names=all_in,
            out_names=tuple(out_names),
            lowering_input_output_aliases=(),
            sim_require_finite=True,
            sim_require_nnan=True,
            nc=nc,
        )
        return tuple(outs)

    devices = jax.devices()[:n_cores]
    mesh = Mesh(np.asarray(devices), ("core",))
    spec = PartitionSpec("core")
    donate = tuple(range(n_params, n_params + n_outs))
    sharded = jax.jit(
        shard_map(
            _body,
            mesh=mesh,
            in_specs=(spec,) * (n_params + n_outs),
            out_specs=(spec,) * n_outs,
            check_rep=False,
        ),
        donate_argnums=donate,
        keep_unused=True,
    )
    sh = NamedSharding(mesh, spec)
    gshapes = [
        (n_cores * av.shape[0], *av.shape[1:]) for av in out_avals
    ]
    gdtypes = [av.dtype for av in out_avals]
    zeros_fn = jax.jit(
        lambda: tuple(jnp.zeros(s, d) for s, d in zip(gshapes, gdtypes)),
        out_shardings=tuple(sh for _ in gshapes),
    )

    dev_cache = {}

    def run(global_inputs):
        args = []
        for name in in_names:
            if name == "masks":
                # call-invariant input: keep it device-resident
                if name not in dev_cache:
                    dev_cache[name] = jax.device_put(global_inputs[name], sh)
                args.append(dev_cache[name])
            else:
                args.append(global_inputs[name])
        zs = zeros_fn()
        outs = sharded(*args, *zs)
        return dict(zip(out_names, outs))

    return run


def _weights_key(inputs):
    import hashlib

    h = hashlib.md5()
    for k in ("qkv_w", "qkv_b", "gate_w", "gate_b", "ln_g", "ln_b"):
        a = np.ascontiguousarray(inputs[k])
        h.update(k.encode())
        h.update(str(a.shape).encode())
        h.update(str(a.dtype).encode())
        h.update(a.tobytes()[::257])
    return h.hexdigest()


def kernel(**inputs):
    prec = "f16"
    key = ("exec", prec, _weights_key(inputs))
    if key not in _CACHE:
        consts = _prep_consts(inputs, prec)
        nc = build_program(prec=prec, consts=consts)
        _CACHE[key] = _build_exec(nc)
    run = _CACHE[key]
    outs = run(_host_globals(inputs, prec))
    return _host_assemble(outs["out"])
